# revision 38
# baseline (speedup 1.0000x reference)
"""FAVOR causal self-attention (Performer) Trainium2 kernel.

Sharding: 8 cores = 2 (batch) x 4 (head groups of 4 heads). Each core
computes qkv for its heads, runs chunked linear attention (L=128), applies
its slice of the output projection, and returns a partial (T, C) output;
partials are summed on the host (+ b_proj broadcast).

Structure:
  phase 1   q+k projections for BOTH mi groups slab-interleaved: per
            arriving x slab the PE does 8x512 columns against the ~1.4us
            slab arrival cadence, staying busy through the input stream
  phase 2   eq/ekt = exp(omega^T {q,k}) m-major, row-tiled head pairs
  phase 2.5 per-chunk precompute, fully pipelined (no serial deps):
            pk = [projk|-nsq] token-major, ekhf = exp(pk + ln 1/16)
            (both heads + fk columns in one activation), vh = [V|1] * fk * c
  phase 3   chunked FAVOR: A/intra/inter/state matmuls + normalize, all
            4 heads batched per vector/scalar op; phase 4 (c_proj tile +
            output DMA per token tile) inlined per chunk.  Token half 0's
            chunks (0-3) are emission-interleaved with half 1's qkv work
            so the favor pipeline starts while x slabs 4-7 still stream.

Layout tricks:
  - k stored per head as ktsq_h (128,T): even heads rows 0:64 = kT,
    64:128 = kT^2; ODD heads swapped so the per-pair omega-projection
    matmuls hit disjoint PE row groups and run concurrently. A row-swapped
    const (on2) recovers [projk|-nsq].  k^2 runs on the idle gpsimd from
    the evicted SBUF k rows (e_k reads only k, so squares stay off the
    scalar exp chain); b_attn is zeros by spec so qkv biases are dropped.
  - v stored as (128, 4*65) with a ones column after each head's 64, so the
    intra and state matmuls take a single (tj,65) moving operand.
  - all DMAs are dense [128,N] copies of host-prearranged images on the two
    HW rings (sync/scalar), issue-ordered by consumption; xtall is
    slab-major so every slab lands as one contiguous [128,1024] copy.
"""
import math
import sys

sys.path.insert(0, "/opt/trn_rl_repo")

import numpy as np

import concourse.bass as bass
import concourse.mybir as mybir
from concourse.tile import TileContext

T, C = 1024, 1024
NH, D, M = 16, 64, 128
L = 128           # chunk length
HPC = 4           # heads per core
NT = T // 128     # 8 token tiles
NK = C // 128     # 8 contraction tiles
F32, F16 = mybir.dt.float32, mybir.dt.float16
LN_SCALE = math.log(1.0 / 16.0)       # folded into the exps
NEG_HALF_LN_M = -0.5 * math.log(M)
VH_SCALE = math.exp(NEG_HALF_LN_M - LN_SCALE)   # vh = [V|1]*fk*VH_SCALE
N_FILL = 12                           # HAM keep-warm filler matmuls

# consts16 column offsets
C16_OM = 0          # omega2 [128,128]
C16_ON = 128        # [omega|0 ; 0|-0.5]  [128,129]
C16_ON2 = 257       # row-swapped variant [128,129]
C16_MK = 386        # causal mask [128,128]
C16_ID = 514        # identity    [128,128]
C16_BV = 642        # unused padding (b_attn zeros); width kept for SBUF layout
C16_W = 898


def _split_waits(nc):
    """Walrus codegen accepts 1 sync wait per instruction (2 on
    EventSemaphore). Tile can emit more; hoist the excess onto
    EventSemaphore instructions inserted immediately before, same engine."""
    for fn in nc.m.functions:
        for bb in fn.blocks:
            insts = bb.instructions
            i = 0
            while i < len(insts):
                inst = insts[i]
                si = inst.sync_info
                if si is None:
                    i += 1
                    continue
                waits = list(si.on_wait or [])
                cap = 2 if isinstance(inst, mybir.InstEventSemaphore) else 1
                if len(waits) <= cap:
                    i += 1
                    continue
                keep, excess = waits[:cap], waits[cap:]
                new_insts = []
                for j in range(0, len(excess), 2):
                    ev = mybir.InstEventSemaphore(
                        name=nc.get_next_instruction_name(),
                        engine=inst.engine,
                        ins=[],
                        outs=[],
                        sync_info=mybir.SyncInfo(
                            on_wait=excess[j:j + 2], on_update=[]),
                    )
                    nc.register_instruction(ev)
                    new_insts.append(ev)
                inst.sync_info = mybir.SyncInfo(
                    on_wait=keep, on_update=list(si.on_update or []))
                for k, ev in enumerate(new_insts):
                    insts.insert(i + k, ev)
                i += len(new_insts) + 1


def build_bass():
    nc = bass.Bass()

    ximg = nc.dram_tensor("ximg", [128, 8 * 1024], F16, kind="ExternalInput")
    wqkk = nc.dram_tensor("wqkk", [128, NK * 256], F16, kind="ExternalInput")
    wqkq = nc.dram_tensor("wqkq", [128, NK * 256], F16, kind="ExternalInput")
    wvimg = nc.dram_tensor("wvimg", [128, NK * 256], F16, kind="ExternalInput")
    wpimg = nc.dram_tensor("wpimg", [128, 2 * C], F16, kind="ExternalInput")
    consts16 = nc.dram_tensor("consts16", [128, C16_W], F16, kind="ExternalInput")
    outp = nc.dram_tensor("outp", [T, C], F16, kind="ExternalOutput")

    Exp = mybir.ActivationFunctionType.Exp
    Square = mybir.ActivationFunctionType.Square
    Mult = mybir.AluOpType.mult

    with TileContext(nc) as tc:
        with (
            tc.tile_pool(name="big", bufs=1) as big,          # resident data
            tc.tile_pool(name="cpy", bufs=6) as cpy,          # staging tiles
            tc.tile_pool(name="chk", bufs=6) as chk,          # chunk tiles
            tc.tile_pool(name="col", bufs=8) as col,          # small columns
            tc.tile_pool(name="ps", bufs=1, space="PSUM") as ps,
        ):
            def bankA():
                return ps.tile([128, 512], F32, name="bankA", bufs=6)

            # ---- resident tiles ----
            c16 = big.tile([128, C16_W], F16, name="c16")
            # xtall is slab-major: slab j = (ki pair j%4, token half j//4),
            # within a slab: [ki_in_pair(2) x 512 tokens].  Every slab DMA is
            # then a fully contiguous [128,1024] copy (128 x 2KB descriptors).
            xtall = big.tile([128, NK * T], F16, name="xtall")
            wqkk_all = big.tile([128, NK * 256], F16, name="wqkk_all")
            wqkq_all = big.tile([128, NK * 256], F16, name="wqkq_all")
            wvall = big.tile([128, NK * 256], F16, name="wvall")
            wpall = big.tile([128, 2 * C], F16, name="wpall")

            def xt(ki, lo, n):
                # token window [lo, lo+n) must stay within one 512-half
                a, b = ki // 2, ki % 2
                th = lo // 512
                off = (a + 4 * th) * 1024 + b * 512 + (lo - th * 512)
                return xtall[:, off:off + n]

            def xslab(j):
                return (xtall[:, j * 1024:(j + 1) * 1024],
                        ximg[:, j * 1024:(j + 1) * 1024])

            # ---- DMA kicks: 2 HW rings (sync, scalar), per-ring order matches
            # consumption: wqkk first (head of phase 1), then x half 0, c16
            # (pre_pk/e_k), wv (v_group), wqkq (q side), x half 1, wp last.
            # Scalar's ring gets a short list so the engine frees early for
            # the phase-1 Square activations.
            nc.sync.dma_start(out=wqkk_all[:, 0:1024], in_=wqkk[:, 0:1024])
            nc.scalar.dma_start(out=wqkk_all[:, 1024:2048],
                                in_=wqkk[:, 1024:2048])
            o, i_ = xslab(0)
            nc.sync.dma_start(out=o, in_=i_)
            o, i_ = xslab(1)
            nc.scalar.dma_start(out=o, in_=i_)
            nc.sync.dma_start(out=wqkq_all[:, 0:1024], in_=wqkq[:, 0:1024])
            nc.scalar.dma_start(out=wqkq_all[:, 1024:2048],
                                in_=wqkq[:, 1024:2048])
            o, i_ = xslab(2)
            nc.sync.dma_start(out=o, in_=i_)
            o, i_ = xslab(3)
            nc.scalar.dma_start(out=o, in_=i_)
            nc.scalar.dma_start(out=c16, in_=consts16[:, :])
            nc.sync.dma_start(out=wvall[:, :], in_=wvimg[:, :])
            for j in (4, 5, 6, 7):
                o, i_ = xslab(j)
                nc.sync.dma_start(out=o, in_=i_)
            nc.sync.dma_start(out=wpall[:, :], in_=wpimg[:, :])

            om_sb = c16[:, C16_OM:C16_OM + 128]
            on_sb = c16[:, C16_ON:C16_ON + 129]
            on2_sb = c16[:, C16_ON2:C16_ON2 + 129]
            mk_sb = c16[:, C16_MK:C16_MK + 128]
            id_sb = c16[:, C16_ID:C16_ID + 128]

            junk = big.tile([128, 128], F16, name="junk")
            nc.vector.memset(junk[0:1, 0:1], 0.0)   # cheapest possible write
            lnsc_sb = big.tile([128, 1], F32, name="lnsc")
            nc.vector.memset(lnsc_sb, LN_SCALE)
            wfill = big.tile([128, 260], F16, name="wfill")
            nc.vector.memset(wfill, 0.0)

            # ---- PE warm-up fillers (results never read) ----
            wps = ps.tile([128, 512], F32, name="pk", bufs=1)
            for wi in range(N_FILL):
                nc.tensor.matmul(wps[:, 0:128], junk[:, :],
                                 junk[:, :], start=True, stop=True)

            # state bank, pre-zeroed so state matmuls accumulate start=False
            sp3 = [big.tile([128, 4 * (D + 1)], F16, name=f"spair{j}")
                   for j in range(3)]
            ps_s = ps.tile([128, 4 * (D + 1)], F32, name="psS", bufs=1)
            nc.tensor.matmul(ps_s[:, :], wfill[:, 0:128], wfill[:, 0:260],
                             start=True, stop=True, skip_group_check=True)

            wv_sb = [wvall[:, ki * HPC * D:(ki + 1) * HPC * D]
                     for ki in range(NK)]
            wp_sb = [wpall[:, ci2 * C:(ci2 + 1) * C] for ci2 in range(2)]

            def kblk(ki, j):
                return wqkk_all[:, ki * 256 + j * 128: ki * 256 + (j + 1) * 128]

            def qblk(ki, j):
                return wqkq_all[:, ki * 256 + j * 128: ki * 256 + (j + 1) * 128]

            # ---- persistent intermediates ----
            qt_sb = [big.tile([128, T], F16, name=f"qt{j}") for j in range(2)]
            ktsq_sb = [big.tile([128, T], F16, name=f"ktsq{h}") for h in range(HPC)]
            eq_sb = [big.tile([128, T], F16, name=f"eq{h}") for h in range(HPC)]
            ekt_sb = [big.tile([128, T], F16, name=f"ekt{h}") for h in range(HPC)]
            v_sb = [big.tile([128, HPC * (D + 1)], F16, name=f"v{ti}")
                    for ti in range(NT)]
            # per-chunk precomputed: ekhf blocks [ekh_h0|fk_h0|ekh_h1|fk_h1]
            ekhf = big.tile([128, 16 * 258], F16, name="ekhf")
            vh_all = [big.tile([128, HPC * (D + 1)], F16, name=f"vh{ti}")
                      for ti in range(NT)]
            yt_all = big.tile([128, 2 * T], F16, name="yt_all")

            # ---- phase 1: qkv projection groups ----
            def qk_evict(mi, ni, p_):
                tsl = slice(ni * 512, (ni + 1) * 512)
                if mi < 2:
                    nc.vector.tensor_copy(qt_sb[mi][:, tsl], p_[:, :])
                else:
                    # b_attn is zeros (spec fill).  k rows evict on two
                    # engines; k^2 computed on the idle gpsimd from the
                    # evicted SBUF copy (e_k reads only the k rows, so the
                    # square is off the exp critical chain).
                    for par in range(2):
                        h = (mi - 2) * 2 + par
                        rs = par * 64          # psum rows holding this head
                        ds = par * 64          # dest rows: k stays in place
                        os = 64 - par * 64     # other rows get k^2
                        eng = nc.vector if par == 0 else nc.scalar
                        if eng is nc.scalar:
                            nc.scalar.copy(
                                ktsq_sb[h][ds:ds + 64, tsl], p_[rs:rs + 64, :])
                        else:
                            nc.vector.tensor_copy(
                                ktsq_sb[h][ds:ds + 64, tsl], p_[rs:rs + 64, :])
                        nc.gpsimd.tensor_tensor(
                            ktsq_sb[h][os:os + 64, tsl],
                            ktsq_sb[h][ds:ds + 64, tsl],
                            ktsq_sb[h][ds:ds + 64, tsl], op=Mult)

            def qk_group(mi, ni):
                p_ = bankA()
                for ki in range(NK):
                    nc.tensor.matmul(
                        p_[:, :],
                        kblk(ki, mi - 2) if mi >= 2 else qblk(ki, mi),
                        xt(ki, ni * 512, 512),
                        start=(ki == 0), stop=(ki == NK - 1))
                qk_evict(mi, ni, p_)

            def phase1_interleaved(ni):
                # all 4 projection groups chase arriving x slabs together;
                # the q-side matmuls run one slab behind the k-side so the
                # stream head needs only wqkk + x0 (wqkq arrives ~1.7us
                # after x0 on the ring and must not stall the pipeline).
                pb = {mi: bankA() for mi in (2, 3, 0, 1)}
                sched = []
                for s in range(4):
                    for mi in (2, 3):
                        sched += [(mi, 2 * s), (mi, 2 * s + 1)]
                    if s >= 1:
                        for mi in (0, 1):
                            sched += [(mi, 2 * (s - 1)), (mi, 2 * s - 1)]
                for mi in (0, 1):
                    sched += [(mi, 6), (mi, 7)]
                total = {mi: sum(1 for m, _ in sched if m == mi)
                         for mi in (0, 1, 2, 3)}
                done = {mi: 0 for mi in total}
                for mi, ki in sched:
                    done[mi] += 1
                    nc.tensor.matmul(
                        pb[mi][:, :],
                        kblk(ki, mi - 2) if mi >= 2 else qblk(ki, mi),
                        xt(ki, ni * 512, 512),
                        start=(done[mi] == 1),
                        stop=(done[mi] == total[mi]))
                for mi in (2, 3, 0, 1):
                    qk_evict(mi, ni, pb[mi])

            # ---- phase 2: exp(omega^T q), exp(omega^T k), row-tiled pairs ----
            def e_q_pair(mi, ni):
                tsl = slice(ni * 512, (ni + 1) * 512)
                banks = []
                for par in range(2):
                    rs = par * 64
                    p_ = bankA()
                    nc.tensor.matmul(p_[:, :], om_sb[rs:rs + 64, :],
                                     qt_sb[mi][rs:rs + 64, tsl],
                                     start=True, stop=True)
                    banks.append(p_)
                for par in range(2):
                    nc.scalar.activation(eq_sb[2 * mi + par][:, tsl],
                                         banks[par][:, :], Exp,
                                         bias=lnsc_sb[:, :], scale=1.0)

            def e_k_pair(pair, ni):
                tsl = slice(ni * 512, (ni + 1) * 512)
                banks = []
                for par in range(2):
                    h, rs = 2 * pair + par, par * 64
                    p_ = bankA()
                    nc.tensor.matmul(p_[:, :], om_sb[rs:rs + 64, :],
                                     ktsq_sb[h][rs:rs + 64, tsl],
                                     start=True, stop=True)
                    banks.append(p_)
                for par in range(2):
                    nc.scalar.activation(ekt_sb[2 * pair + par][:, tsl],
                                         banks[par][:, :], Exp,
                                         bias=lnsc_sb[:, :], scale=1.0)

            def v_group(ti):
                nc.vector.memset(
                    v_sb[ti][:, :].rearrange("p (h c) -> p h c", c=D + 1)
                    [:, :, D:D + 1], 1.0)
                p_ = bankA()
                for ki in range(NK):
                    nc.tensor.matmul(
                        p_[:, 0:HPC * D],
                        xt(ki, ti * 128, 128),
                        wv_sb[ki][:, :],
                        start=(ki == 0), stop=(ki == NK - 1))
                nc.vector.tensor_copy(
                    v_sb[ti][:, :].rearrange("p (h c) -> p h c", c=D + 1)
                    [:, :, 0:D],
                    p_[:, 0:HPC * D].rearrange("p (h c) -> p h c", c=D))

            # ---- phase 2.5: per-chunk ekh/fk/vh precompute (pipelined) ----
            def chunk_pre_pk(ci, pair):
                h0, h1 = 2 * pair, 2 * pair + 1
                b = pair * NT + ci
                csl = slice(ci * L, (ci + 1) * L)
                pk = ps.tile([128, 512], F32, name="pk", bufs=1)
                nc.tensor.matmul(pk[:, 0:129], ktsq_sb[h0][:, csl],
                                 on_sb[:, :], start=True, stop=True,
                                 skip_group_check=True)
                nc.tensor.matmul(pk[:, 129:258], ktsq_sb[h1][:, csl],
                                 on2_sb[:, :], start=False, stop=True,
                                 skip_group_check=True)
                # exp over [projk|-nsq] for both heads: ekh + fk in one op
                nc.scalar.activation(
                    ekhf[:, b * 258:(b + 1) * 258]
                    .rearrange("p (a c) -> p a c", a=2),
                    pk[:, 0:258].rearrange("p (a c) -> p a c", a=2),
                    Exp, bias=lnsc_sb[:, :], scale=1.0)

            def chunk_pre_vh(ci, pair):
                h0, h1 = 2 * pair, 2 * pair + 1
                b = pair * NT + ci
                fk0 = ekhf[:, b * 258 + 128:b * 258 + 129]
                fk_b = bass.AP(tensor=fk0.tensor, offset=fk0.offset,
                               ap=[fk0.ap[0], [129, 2], [0, D + 1]])
                nc.vector.scalar_tensor_tensor(
                    vh_all[ci][:, h0 * (D + 1):(h1 + 1) * (D + 1)]
                    .rearrange("p (a c) -> p a c", a=2),
                    v_sb[ci][:, h0 * (D + 1):(h1 + 1) * (D + 1)]
                    .rearrange("p (a c) -> p a c", a=2),
                    VH_SCALE, fk_b, op0=Mult, op1=Mult)

            # ---- phase 3: chunked FAVOR, 3-stage software pipeline ----
            # A(ci): pA matmuls -> atm (vector), state matmuls, spair copy
            # B(ci): inter/intra matmuls into pY -> rc4, ych (vector)
            # C(ci): transposes -> yt copy, c_proj tile, output DMA
            # Emitted as A(c), B(c-1), C(c-2) so every PE op only consumes
            # results produced >= 1 cycle earlier (no PE stalls on vector).
            atm_t = {}
            ych_t = {}
            pyt_t = {}

            def favor_A(ci):
                csl = slice(ci * L, (ci + 1) * L)
                pA = bankA()
                for h in range(HPC):
                    nc.tensor.matmul(pA[:, h * 128:(h + 1) * 128],
                                     ekt_sb[h][:, csl], eq_sb[h][:, csl],
                                     start=(h == 0), stop=True,
                                     skip_group_check=True)
                atm = chk.tile([128, 512], F16, name="atm")
                atm_t[ci] = atm
                mk_b = bass.AP(
                    tensor=mk_sb.tensor, offset=mk_sb.offset,
                    ap=[mk_sb.ap[0], [0, HPC], mk_sb.ap[1]])
                nc.vector.tensor_tensor(
                    atm[:, :].rearrange("p (a c) -> p a c", a=HPC),
                    pA[:, :].rearrange("p (a c) -> p a c", a=HPC),
                    mk_b, op=Mult)
                # state update (bank pre-zeroed: accumulate with start=False)
                for h in range(HPC):
                    b = (h // 2) * NT + ci
                    ssl = h * (D + 1)
                    nc.tensor.matmul(
                        ps_s[:, ssl:ssl + D + 1],
                        ekhf[:, b * 258 + (h % 2) * 129:
                             b * 258 + (h % 2) * 129 + 128],
                        vh_all[ci][:, ssl:ssl + D + 1],
                        start=False, stop=(ci == NT - 1),
                        skip_group_check=True)
                if ci < NT - 1:
                    nc.scalar.copy(sp3[ci % 3][:, :], ps_s[:, :])

            def favor_B(ci):
                csl = slice(ci * L, (ci + 1) * L)
                atm = atm_t.pop(ci)
                pY = bankA()
                for h in range(HPC):
                    ysl = slice(h * (D + 1), (h + 1) * (D + 1))
                    if ci > 0:
                        nc.tensor.matmul(
                            pY[:, ysl], eq_sb[h][:, csl],
                            sp3[(ci - 1) % 3][:, ysl],
                            start=(h == 0), stop=True,
                            skip_group_check=True)
                    nc.tensor.matmul(
                        pY[:, ysl], atm[:, h * 128:(h + 1) * 128],
                        vh_all[ci][:, ysl],
                        start=(ci == 0 and h == 0), stop=True,
                        skip_group_check=True)
                rc4 = col.tile([128, HPC], F32, name="rc4")
                nc.vector.reciprocal(
                    rc4,
                    pY[:, 0:HPC * (D + 1)]
                    .rearrange("p (a c) -> p a c", a=HPC)
                    [:, :, D:D + 1].rearrange("p a c -> p (a c)"))
                ych = chk.tile([128, 256], F16, name="ych")
                ych_t[ci] = ych
                rc_b = bass.AP(
                    tensor=rc4.tensor, offset=rc4.offset,
                    ap=[rc4.ap[0], rc4.ap[1], [0, D]])
                nc.vector.tensor_tensor(
                    ych[:, :].rearrange("p (a c) -> p a c", a=HPC),
                    pY[:, 0:HPC * (D + 1)]
                    .rearrange("p (a c) -> p a c", a=HPC)[:, :, 0:D],
                    rc_b, op=Mult)

            def favor_C1(ci):
                ych = ych_t.pop(ci)
                pyt = ps.tile([128, 256], F16, name="bankA", bufs=6)
                nc.tensor.matmul(pyt[:, 0:128], ych[:, 0:128], id_sb[:, :],
                                 is_transpose=True, start=True, stop=True,
                                 skip_group_check=True)
                nc.tensor.matmul(pyt[:, 128:256], ych[:, 128:256],
                                 id_sb[:, :], is_transpose=True,
                                 start=False, stop=True,
                                 skip_group_check=True)
                nc.scalar.copy(
                    yt_all[:, :].rearrange("p (a t) -> p a t", a=2)
                    [:, :, ci * 128:(ci + 1) * 128],
                    pyt[:, :].rearrange("p (a c) -> p a c", a=2))

            def favor_C2(ci):
                # ---- phase 4 for this token tile ----
                osb = cpy.tile([128, 1024], F16, name="osb")
                last = ci == NT - 1
                for ni in range(2):
                    nsl = slice(ni * 512, (ni + 1) * 512)
                    pp = bankA()
                    for ci2 in range(2):
                        nc.tensor.matmul(
                            pp[:, :],
                            yt_all[:, ci2 * T + ci * 128:
                                   ci2 * T + (ci + 1) * 128],
                            wp_sb[ci2][:, nsl],
                            start=(ci2 == 0), stop=(ci2 == 1))
                    if (ni == 0) != last:
                        nc.scalar.copy(osb[:, nsl], pp[:, :])
                    else:
                        nc.vector.tensor_copy(osb[:, nsl], pp[:, :])
                # one contiguous [128, 1024] drain per token tile
                keng = nc.scalar if last else nc.sync
                keng.dma_start(
                    out=outp[ci * 128:(ci + 1) * 128, :], in_=osb[:, :])

            # ---- program order ----
            # Token half 0 (chunks 0-3) is fully independent of half 1, so
            # its FAVOR chunks are interleaved with half 1's qkv/exp phases:
            # the favor pipeline starts as soon as half 0 is projected while
            # x slabs 4-7 are still arriving.
            phase1_interleaved(0)
            e_k_pair(0, 0)
            e_q_pair(0, 0)
            e_k_pair(1, 0)
            e_q_pair(1, 0)
            for ci in range(4):
                chunk_pre_pk(ci, 0)
                chunk_pre_pk(ci, 1)
            v_group(0)
            chunk_pre_vh(0, 0)
            chunk_pre_vh(0, 1)
            qk_group(2, 1)
            favor_A(0)
            v_group(1)
            chunk_pre_vh(1, 0)
            chunk_pre_vh(1, 1)
            qk_group(3, 1)
            favor_A(1)
            favor_B(0)
            v_group(2)
            chunk_pre_vh(2, 0)
            chunk_pre_vh(2, 1)
            chunk_pre_pk(4, 0)
            chunk_pre_pk(4, 1)
            e_k_pair(0, 1)
            e_k_pair(1, 1)
            favor_A(2)
            favor_B(1)
            favor_C1(0)
            v_group(3)
            chunk_pre_vh(3, 0)
            chunk_pre_vh(3, 1)
            qk_group(0, 1)
            qk_group(1, 1)
            favor_A(3)
            favor_B(2)
            favor_C1(1)
            favor_C2(0)
            e_q_pair(0, 1)
            e_q_pair(1, 1)
            for ci in range(5, 8):
                chunk_pre_pk(ci, 0)
                chunk_pre_pk(ci, 1)
            favor_B(3)
            favor_C1(2)
            favor_C2(1)
            for ti in range(4, 8):
                v_group(ti)
            for ci in range(4, 8):
                chunk_pre_vh(ci, 0)
                chunk_pre_vh(ci, 1)
            favor_C1(3)
            favor_C2(2)
            favor_A(4)
            favor_C2(3)
            favor_A(5)
            favor_B(4)
            favor_A(6)
            favor_B(5)
            favor_C1(4)
            favor_A(7)
            favor_B(6)
            favor_C1(5)
            favor_C2(4)
            favor_B(7)
            favor_C1(6)
            favor_C2(5)
            favor_C1(7)
            favor_C2(6)
            favor_C2(7)

    _split_waits(nc)
    return nc


_NC_CACHE = None


def _get_nc():
    global _NC_CACHE
    if _NC_CACHE is None:
        _NC_CACHE = build_bass()
    return _NC_CACHE


def _img8(w):
    # [1024, n] -> [128, 8*n] with 128-row blocks laid side by side
    n = w.shape[1]
    return np.ascontiguousarray(
        w.reshape(8, 128, n).transpose(1, 0, 2).reshape(128, 8 * n))


def kernel(x, W_attn, b_attn, W_proj, b_proj, omega):
    from concourse.bass_utils import run_bass_kernel_spmd

    x = np.asarray(x, dtype=np.float32)
    W_attn = np.asarray(W_attn, dtype=np.float32)
    b_attn = np.asarray(b_attn, dtype=np.float32)
    W_proj = np.asarray(W_proj, dtype=np.float32)
    b_proj = np.asarray(b_proj, dtype=np.float32)
    omega = np.asarray(omega, dtype=np.float32)

    B = x.shape[0]
    scale = 1.0 / math.sqrt(D)
    omega2 = np.concatenate([omega, omega], axis=0)
    omnsq = np.zeros((128, 129), np.float32)
    omnsq[0:64, 0:128] = omega
    omnsq[64:128, 128] = -0.5
    omnsq2 = np.zeros((128, 129), np.float32)
    omnsq2[64:128, 0:128] = omega
    omnsq2[0:64, 128] = -0.5
    maskT = np.triu(np.ones((128, 128), np.float32))
    ident = np.eye(128, dtype=np.float32)
    c16_base = np.zeros((128, C16_W), np.float16)
    c16_base[:, C16_OM:C16_OM + 128] = omega2
    c16_base[:, C16_ON:C16_ON + 129] = omnsq
    c16_base[:, C16_ON2:C16_ON2 + 129] = omnsq2
    c16_base[:, C16_MK:C16_MK + 128] = maskT
    c16_base[:, C16_ID:C16_ID + 128] = ident
    c16_base = np.ascontiguousarray(c16_base)

    # x images: 8 slabs of (ki pair, token half), token half 0 first
    ximgs = []
    for b in range(B):
        xT = np.ascontiguousarray(x[b].T).astype(np.float16)
        Xr = xT.reshape(8, 128, 1024)
        slabs = []
        for j in range(8):
            kp, th = (j % 4) * 2, j // 4
            s = Xr[kp:kp + 2, :, th * 512:(th + 1) * 512]   # [2,128,512]
            slabs.append(s.transpose(1, 0, 2).reshape(128, 1024))
        ximgs.append(np.ascontiguousarray(np.concatenate(slabs, axis=1)))

    # b_attn is zeros by the problem spec, so qkv biases are dropped on
    # device; b_proj is added on the host below.
    in_maps = []
    for core in range(8):
        b, g = core // 4, core % 4
        ch0 = g * HPC * D
        wq = (W_attn[:, ch0:ch0 + HPC * D] * scale).astype(np.float16)
        wk = (W_attn[:, C + ch0:C + ch0 + HPC * D] * scale).astype(np.float16)
        wv_ = W_attn[:, 2 * C + ch0:2 * C + ch0 + HPC * D].astype(np.float16)
        wp_ = W_proj[ch0:ch0 + HPC * D, :].astype(np.float16)
        in_maps.append({
            "ximg": ximgs[b],
            "wqkk": _img8(wk), "wqkq": _img8(wq), "wvimg": _img8(wv_),
            "wpimg": np.ascontiguousarray(
                wp_.reshape(2, 128, 1024).transpose(1, 0, 2)
                .reshape(128, 2048)),
            "consts16": c16_base,
        })

    nc = _get_nc()
    res = run_bass_kernel_spmd(nc, in_maps, list(range(8)))

    out = np.zeros((B, T, C), dtype=np.float32)
    for core in range(8):
        out[core // 4] += res.results[core]["outp"]
    out += b_proj[None, None, :]
    return out



# revision 39
# speedup vs baseline: 1.0418x; 1.0418x over previous
"""FAVOR causal self-attention (Performer) Trainium2 kernel.

Sharding: 8 cores = 2 (batch) x 4 (head groups of 4 heads). Each core
computes qkv for its heads, runs chunked linear attention (L=128), applies
its slice of the output projection, and returns a partial (T, C) output;
partials are summed on the host (+ b_proj broadcast).

Structure:
  phase 1   q+k projections for BOTH mi groups slab-interleaved: per
            arriving x slab the PE does 8x512 columns against the ~1.4us
            slab arrival cadence, staying busy through the input stream
  phase 2   eq/ekt = exp(omega^T {q,k}) m-major, row-tiled head pairs
  phase 2.5 per-chunk precompute, fully pipelined (no serial deps):
            pk = [projk|-nsq] token-major, ekhf = exp(pk + ln 1/16)
            (both heads + fk columns in one activation), vh = [V|1] * fk * c
  phase 3   chunked FAVOR: A/intra/inter/state matmuls + normalize, all
            4 heads batched per vector/scalar op; phase 4 (c_proj tile +
            output DMA per token tile) inlined per chunk.  Token half 0's
            chunks (0-3) are emission-interleaved with half 1's qkv work
            so the favor pipeline starts while x slabs 4-7 still stream.

Layout tricks:
  - k stored per head as ktsq_h (128,T): even heads rows 0:64 = kT,
    64:128 = kT^2; ODD heads swapped so the per-pair omega-projection
    matmuls hit disjoint PE row groups and run concurrently. A row-swapped
    const (on2) recovers [projk|-nsq].  k^2 runs on the idle gpsimd from
    the evicted SBUF k rows (e_k reads only k, so squares stay off the
    scalar exp chain); b_attn is zeros by spec so qkv biases are dropped.
  - v stored as (128, 4*65) with a ones column after each head's 64, so the
    intra and state matmuls take a single (tj,65) moving operand.
  - all DMAs are dense [128,N] copies of host-prearranged images on the two
    HW rings (sync/scalar), issue-ordered by consumption; xtall is
    slab-major so every slab lands as one contiguous [128,1024] copy.
"""
import math
import sys

sys.path.insert(0, "/opt/trn_rl_repo")

import numpy as np

import concourse.bass as bass
import concourse.mybir as mybir
from concourse.tile import TileContext

T, C = 1024, 1024
NH, D, M = 16, 64, 128
L = 128           # chunk length
HPC = 4           # heads per core
NT = T // 128     # 8 token tiles
NK = C // 128     # 8 contraction tiles
F32, F16 = mybir.dt.float32, mybir.dt.float16
LN_SCALE = math.log(1.0 / 16.0)       # folded into the exps
NEG_HALF_LN_M = -0.5 * math.log(M)
VH_SCALE = math.exp(NEG_HALF_LN_M - LN_SCALE)   # vh = [V|1]*fk*VH_SCALE
N_FILL = 26                           # HAM keep-warm filler matmuls

# consts16 column offsets
C16_OM = 0          # omega2 [128,128]
C16_ON = 128        # [omega|0 ; 0|-0.5]  [128,129]
C16_ON2 = 257       # row-swapped variant [128,129]
C16_MK = 386        # causal mask [128,128]
C16_ID = 514        # identity    [128,128]
C16_BV = 642        # unused padding (b_attn zeros); width kept for SBUF layout
C16_W = 898


def _split_waits(nc):
    """Walrus codegen accepts 1 sync wait per instruction (2 on
    EventSemaphore). Tile can emit more; hoist the excess onto
    EventSemaphore instructions inserted immediately before, same engine."""
    for fn in nc.m.functions:
        for bb in fn.blocks:
            insts = bb.instructions
            i = 0
            while i < len(insts):
                inst = insts[i]
                si = inst.sync_info
                if si is None:
                    i += 1
                    continue
                waits = list(si.on_wait or [])
                cap = 2 if isinstance(inst, mybir.InstEventSemaphore) else 1
                if len(waits) <= cap:
                    i += 1
                    continue
                keep, excess = waits[:cap], waits[cap:]
                new_insts = []
                for j in range(0, len(excess), 2):
                    ev = mybir.InstEventSemaphore(
                        name=nc.get_next_instruction_name(),
                        engine=inst.engine,
                        ins=[],
                        outs=[],
                        sync_info=mybir.SyncInfo(
                            on_wait=excess[j:j + 2], on_update=[]),
                    )
                    nc.register_instruction(ev)
                    new_insts.append(ev)
                inst.sync_info = mybir.SyncInfo(
                    on_wait=keep, on_update=list(si.on_update or []))
                for k, ev in enumerate(new_insts):
                    insts.insert(i + k, ev)
                i += len(new_insts) + 1


def build_bass():
    nc = bass.Bass()

    ximg = nc.dram_tensor("ximg", [128, 8 * 1024], F16, kind="ExternalInput")
    wqkk = nc.dram_tensor("wqkk", [128, NK * 256], F16, kind="ExternalInput")
    wqkq = nc.dram_tensor("wqkq", [128, NK * 256], F16, kind="ExternalInput")
    wvimg = nc.dram_tensor("wvimg", [128, NK * 256], F16, kind="ExternalInput")
    wpimg = nc.dram_tensor("wpimg", [128, 2 * C], F16, kind="ExternalInput")
    consts16 = nc.dram_tensor("consts16", [128, C16_W], F16, kind="ExternalInput")
    outp = nc.dram_tensor("outp", [T, C], F16, kind="ExternalOutput")

    Exp = mybir.ActivationFunctionType.Exp
    Square = mybir.ActivationFunctionType.Square
    Mult = mybir.AluOpType.mult

    with TileContext(nc) as tc:
        with (
            tc.tile_pool(name="big", bufs=1) as big,          # resident data
            tc.tile_pool(name="cpy", bufs=8) as cpy,          # staging tiles
            tc.tile_pool(name="chk", bufs=6) as chk,          # chunk tiles
            tc.tile_pool(name="col", bufs=8) as col,          # small columns
            tc.tile_pool(name="ps", bufs=1, space="PSUM") as ps,
        ):
            def bankA():
                return ps.tile([128, 512], F32, name="bankA", bufs=6)

            # ---- resident tiles ----
            c16 = big.tile([128, C16_W], F16, name="c16")
            # xtall is slab-major: slab j = (ki pair j%4, token half j//4),
            # within a slab: [ki_in_pair(2) x 512 tokens].  Every slab DMA is
            # then a fully contiguous [128,1024] copy (128 x 2KB descriptors).
            xtall = big.tile([128, NK * T], F16, name="xtall")
            wqkk_all = big.tile([128, NK * 256], F16, name="wqkk_all")
            wqkq_all = big.tile([128, NK * 256], F16, name="wqkq_all")
            wvall = big.tile([128, NK * 256], F16, name="wvall")
            wpall = big.tile([128, 2 * C], F16, name="wpall")

            def xt(ki, lo, n):
                # token window [lo, lo+n) must stay within one 512-half
                a, b = ki // 2, ki % 2
                th = lo // 512
                off = (a + 4 * th) * 1024 + b * 512 + (lo - th * 512)
                return xtall[:, off:off + n]

            def xslab(j):
                return (xtall[:, j * 1024:(j + 1) * 1024],
                        ximg[:, j * 1024:(j + 1) * 1024])

            # ---- DMA kicks: 2 HW rings (sync, scalar), per-ring order matches
            # consumption: wqkk first (head of phase 1), then x half 0, c16
            # (pre_pk/e_k), wv (v_group), wqkq (q side), x half 1, wp last.
            # Scalar's ring gets a short list so the engine frees early for
            # the phase-1 Square activations.
            nc.sync.dma_start(out=wqkk_all[:, 0:1024], in_=wqkk[:, 0:1024])
            nc.scalar.dma_start(out=wqkk_all[:, 1024:2048],
                                in_=wqkk[:, 1024:2048])
            nc.sync.dma_start(out=xtall[:, 0:512], in_=ximg[:, 0:512])
            nc.sync.dma_start(out=xtall[:, 512:1024], in_=ximg[:, 512:1024])
            o, i_ = xslab(1)
            nc.scalar.dma_start(out=o, in_=i_)
            nc.sync.dma_start(out=wqkq_all[:, 0:1024], in_=wqkq[:, 0:1024])
            nc.scalar.dma_start(out=wqkq_all[:, 1024:2048],
                                in_=wqkq[:, 1024:2048])
            o, i_ = xslab(2)
            nc.sync.dma_start(out=o, in_=i_)
            o, i_ = xslab(3)
            nc.scalar.dma_start(out=o, in_=i_)
            nc.scalar.dma_start(out=c16, in_=consts16[:, :])
            nc.sync.dma_start(out=wvall[:, :], in_=wvimg[:, :])
            for j in (4, 5, 6, 7):
                o, i_ = xslab(j)
                nc.sync.dma_start(out=o, in_=i_)
            nc.sync.dma_start(out=wpall[:, :], in_=wpimg[:, :])

            om_sb = c16[:, C16_OM:C16_OM + 128]
            on_sb = c16[:, C16_ON:C16_ON + 129]
            on2_sb = c16[:, C16_ON2:C16_ON2 + 129]
            mk_sb = c16[:, C16_MK:C16_MK + 128]
            id_sb = c16[:, C16_ID:C16_ID + 128]

            junk = big.tile([128, 128], F16, name="junk")
            nc.vector.memset(junk[0:1, 0:1], 0.0)   # cheapest possible write
            lnsc_sb = big.tile([128, 1], F32, name="lnsc")
            nc.vector.memset(lnsc_sb, LN_SCALE)
            wfill = big.tile([128, 260], F16, name="wfill")
            nc.vector.memset(wfill, 0.0)

            # ---- PE warm-up fillers (results never read) ----
            wps = ps.tile([128, 512], F32, name="pk", bufs=1)
            for wi in range(N_FILL):
                nc.tensor.matmul(wps[:, 0:128], junk[:, :],
                                 junk[:, :], start=True, stop=True)

            # state bank, pre-zeroed so state matmuls accumulate start=False
            sp3 = [big.tile([128, 4 * (D + 1)], F16, name=f"spair{j}")
                   for j in range(3)]
            ps_s = ps.tile([128, 4 * (D + 1)], F32, name="psS", bufs=1)
            nc.tensor.matmul(ps_s[:, :], wfill[:, 0:128], wfill[:, 0:260],
                             start=True, stop=True, skip_group_check=True)

            wv_sb = [wvall[:, ki * HPC * D:(ki + 1) * HPC * D]
                     for ki in range(NK)]
            wp_sb = [wpall[:, ci2 * C:(ci2 + 1) * C] for ci2 in range(2)]

            def kblk(ki, j):
                return wqkk_all[:, ki * 256 + j * 128: ki * 256 + (j + 1) * 128]

            def qblk(ki, j):
                return wqkq_all[:, ki * 256 + j * 128: ki * 256 + (j + 1) * 128]

            # ---- persistent intermediates ----
            qt_sb = [big.tile([128, T], F16, name=f"qt{j}") for j in range(2)]
            ktsq_sb = [big.tile([128, T], F16, name=f"ktsq{h}") for h in range(HPC)]
            eq_sb = [big.tile([128, T], F16, name=f"eq{h}") for h in range(HPC)]
            ekt_sb = [big.tile([128, T], F16, name=f"ekt{h}") for h in range(HPC)]
            v_sb = [big.tile([128, HPC * (D + 1)], F16, name=f"v{ti}")
                    for ti in range(NT)]
            # per-chunk precomputed: ekhf blocks [ekh_h0|fk_h0|ekh_h1|fk_h1]
            ekhf = big.tile([128, 16 * 258], F16, name="ekhf")
            vh_all = [big.tile([128, HPC * (D + 1)], F16, name=f"vh{ti}")
                      for ti in range(NT)]
            yt_all = big.tile([128, 2 * T], F16, name="yt_all")

            # ---- phase 1: qkv projection groups ----
            def qk_evict(mi, ni, p_):
                tsl = slice(ni * 512, (ni + 1) * 512)
                if mi < 2:
                    nc.vector.tensor_copy(qt_sb[mi][:, tsl], p_[:, :])
                else:
                    # b_attn is zeros (spec fill).  k rows evict on two
                    # engines; k^2 computed on the idle gpsimd from the
                    # evicted SBUF copy (e_k reads only the k rows, so the
                    # square is off the exp critical chain).
                    for par in range(2):
                        h = (mi - 2) * 2 + par
                        rs = par * 64          # psum rows holding this head
                        ds = par * 64          # dest rows: k stays in place
                        os = 64 - par * 64     # other rows get k^2
                        eng = nc.vector if par == 0 else nc.scalar
                        if eng is nc.scalar:
                            nc.scalar.copy(
                                ktsq_sb[h][ds:ds + 64, tsl], p_[rs:rs + 64, :])
                        else:
                            nc.vector.tensor_copy(
                                ktsq_sb[h][ds:ds + 64, tsl], p_[rs:rs + 64, :])
                        nc.gpsimd.tensor_tensor(
                            ktsq_sb[h][os:os + 64, tsl],
                            ktsq_sb[h][ds:ds + 64, tsl],
                            ktsq_sb[h][ds:ds + 64, tsl], op=Mult)

            def qk_group(mi, ni):
                p_ = bankA()
                for ki in range(NK):
                    nc.tensor.matmul(
                        p_[:, :],
                        kblk(ki, mi - 2) if mi >= 2 else qblk(ki, mi),
                        xt(ki, ni * 512, 512),
                        start=(ki == 0), stop=(ki == NK - 1))
                qk_evict(mi, ni, p_)

            def phase1_interleaved(ni):
                # all 4 projection groups chase arriving x slabs together;
                # the q-side matmuls run one slab behind the k-side so the
                # stream head needs only wqkk + x0 (wqkq arrives ~1.7us
                # after x0 on the ring and must not stall the pipeline).
                pb = {mi: bankA() for mi in (2, 3, 0, 1)}
                sched = []
                for s in range(4):
                    for mi in (2, 3):
                        sched += [(mi, 2 * s), (mi, 2 * s + 1)]
                    if s >= 1:
                        for mi in (0, 1):
                            sched += [(mi, 2 * (s - 1)), (mi, 2 * s - 1)]
                for mi in (0, 1):
                    sched += [(mi, 6), (mi, 7)]
                total = {mi: sum(1 for m, _ in sched if m == mi)
                         for mi in (0, 1, 2, 3)}
                done = {mi: 0 for mi in total}
                for mi, ki in sched:
                    done[mi] += 1
                    nc.tensor.matmul(
                        pb[mi][:, :],
                        kblk(ki, mi - 2) if mi >= 2 else qblk(ki, mi),
                        xt(ki, ni * 512, 512),
                        start=(done[mi] == 1),
                        stop=(done[mi] == total[mi]))
                for mi in (2, 3, 0, 1):
                    qk_evict(mi, ni, pb[mi])

            # ---- phase 2: exp(omega^T q), exp(omega^T k), row-tiled pairs ----
            def e_q_pair(mi, ni):
                tsl = slice(ni * 512, (ni + 1) * 512)
                banks = []
                for par in range(2):
                    rs = par * 64
                    p_ = bankA()
                    nc.tensor.matmul(p_[:, :], om_sb[rs:rs + 64, :],
                                     qt_sb[mi][rs:rs + 64, tsl],
                                     start=True, stop=True)
                    banks.append(p_)
                for par in range(2):
                    nc.scalar.activation(eq_sb[2 * mi + par][:, tsl],
                                         banks[par][:, :], Exp,
                                         bias=lnsc_sb[:, :], scale=1.0)

            def e_k_pair(pair, ni):
                tsl = slice(ni * 512, (ni + 1) * 512)
                banks = []
                for par in range(2):
                    h, rs = 2 * pair + par, par * 64
                    p_ = bankA()
                    nc.tensor.matmul(p_[:, :], om_sb[rs:rs + 64, :],
                                     ktsq_sb[h][rs:rs + 64, tsl],
                                     start=True, stop=True)
                    banks.append(p_)
                for par in range(2):
                    nc.scalar.activation(ekt_sb[2 * pair + par][:, tsl],
                                         banks[par][:, :], Exp,
                                         bias=lnsc_sb[:, :], scale=1.0)

            def v_group(ti):
                nc.vector.memset(
                    v_sb[ti][:, :].rearrange("p (h c) -> p h c", c=D + 1)
                    [:, :, D:D + 1], 1.0)
                p_ = bankA()
                for ki in range(NK):
                    nc.tensor.matmul(
                        p_[:, 0:HPC * D],
                        xt(ki, ti * 128, 128),
                        wv_sb[ki][:, :],
                        start=(ki == 0), stop=(ki == NK - 1))
                nc.vector.tensor_copy(
                    v_sb[ti][:, :].rearrange("p (h c) -> p h c", c=D + 1)
                    [:, :, 0:D],
                    p_[:, 0:HPC * D].rearrange("p (h c) -> p h c", c=D))

            # ---- phase 2.5: per-chunk ekh/fk/vh precompute (pipelined) ----
            def chunk_pre_pk(ci, pair):
                h0, h1 = 2 * pair, 2 * pair + 1
                b = pair * NT + ci
                csl = slice(ci * L, (ci + 1) * L)
                pk = ps.tile([128, 512], F32, name="pk", bufs=1)
                nc.tensor.matmul(pk[:, 0:129], ktsq_sb[h0][:, csl],
                                 on_sb[:, :], start=True, stop=True,
                                 skip_group_check=True)
                nc.tensor.matmul(pk[:, 129:258], ktsq_sb[h1][:, csl],
                                 on2_sb[:, :], start=False, stop=True,
                                 skip_group_check=True)
                # exp over [projk|-nsq] for both heads: ekh + fk in one op
                nc.scalar.activation(
                    ekhf[:, b * 258:(b + 1) * 258]
                    .rearrange("p (a c) -> p a c", a=2),
                    pk[:, 0:258].rearrange("p (a c) -> p a c", a=2),
                    Exp, bias=lnsc_sb[:, :], scale=1.0)

            def chunk_pre_vh(ci, pair):
                h0, h1 = 2 * pair, 2 * pair + 1
                b = pair * NT + ci
                fk0 = ekhf[:, b * 258 + 128:b * 258 + 129]
                fk_b = bass.AP(tensor=fk0.tensor, offset=fk0.offset,
                               ap=[fk0.ap[0], [129, 2], [0, D + 1]])
                nc.vector.scalar_tensor_tensor(
                    vh_all[ci][:, h0 * (D + 1):(h1 + 1) * (D + 1)]
                    .rearrange("p (a c) -> p a c", a=2),
                    v_sb[ci][:, h0 * (D + 1):(h1 + 1) * (D + 1)]
                    .rearrange("p (a c) -> p a c", a=2),
                    VH_SCALE, fk_b, op0=Mult, op1=Mult)

            # ---- phase 3: chunked FAVOR, 3-stage software pipeline ----
            # A(ci): pA matmuls -> atm (vector), state matmuls, spair copy
            # B(ci): inter/intra matmuls into pY -> rc4, ych (vector)
            # C(ci): transposes -> yt copy, c_proj tile, output DMA
            # Emitted as A(c), B(c-1), C(c-2) so every PE op only consumes
            # results produced >= 1 cycle earlier (no PE stalls on vector).
            atm_t = {}
            ych_t = {}
            pyt_t = {}

            def favor_A(ci):
                csl = slice(ci * L, (ci + 1) * L)
                pA = bankA()
                for h in range(HPC):
                    nc.tensor.matmul(pA[:, h * 128:(h + 1) * 128],
                                     ekt_sb[h][:, csl], eq_sb[h][:, csl],
                                     start=(h == 0), stop=True,
                                     skip_group_check=True)
                atm = chk.tile([128, 512], F16, name="atm")
                atm_t[ci] = atm
                mk_b = bass.AP(
                    tensor=mk_sb.tensor, offset=mk_sb.offset,
                    ap=[mk_sb.ap[0], [0, HPC], mk_sb.ap[1]])
                nc.vector.tensor_tensor(
                    atm[:, :].rearrange("p (a c) -> p a c", a=HPC),
                    pA[:, :].rearrange("p (a c) -> p a c", a=HPC),
                    mk_b, op=Mult)
                # state update (bank pre-zeroed: accumulate with start=False)
                for h in range(HPC):
                    b = (h // 2) * NT + ci
                    ssl = h * (D + 1)
                    nc.tensor.matmul(
                        ps_s[:, ssl:ssl + D + 1],
                        ekhf[:, b * 258 + (h % 2) * 129:
                             b * 258 + (h % 2) * 129 + 128],
                        vh_all[ci][:, ssl:ssl + D + 1],
                        start=False, stop=(ci == NT - 1),
                        skip_group_check=True)
                if ci < NT - 1:
                    nc.scalar.copy(sp3[ci % 3][:, :], ps_s[:, :])

            def favor_B(ci):
                csl = slice(ci * L, (ci + 1) * L)
                atm = atm_t.pop(ci)
                pY = bankA()
                for h in range(HPC):
                    ysl = slice(h * (D + 1), (h + 1) * (D + 1))
                    if ci > 0:
                        nc.tensor.matmul(
                            pY[:, ysl], eq_sb[h][:, csl],
                            sp3[(ci - 1) % 3][:, ysl],
                            start=(h == 0), stop=True,
                            skip_group_check=True)
                    nc.tensor.matmul(
                        pY[:, ysl], atm[:, h * 128:(h + 1) * 128],
                        vh_all[ci][:, ysl],
                        start=(ci == 0 and h == 0), stop=True,
                        skip_group_check=True)
                rc4 = col.tile([128, HPC], F32, name="rc4")
                nc.vector.reciprocal(
                    rc4,
                    pY[:, 0:HPC * (D + 1)]
                    .rearrange("p (a c) -> p a c", a=HPC)
                    [:, :, D:D + 1].rearrange("p a c -> p (a c)"))
                ych = chk.tile([128, 256], F16, name="ych")
                ych_t[ci] = ych
                rc_b = bass.AP(
                    tensor=rc4.tensor, offset=rc4.offset,
                    ap=[rc4.ap[0], rc4.ap[1], [0, D]])
                nc.vector.tensor_tensor(
                    ych[:, :].rearrange("p (a c) -> p a c", a=HPC),
                    pY[:, 0:HPC * (D + 1)]
                    .rearrange("p (a c) -> p a c", a=HPC)[:, :, 0:D],
                    rc_b, op=Mult)

            def favor_C1(ci):
                ych = ych_t.pop(ci)
                pyt = ps.tile([128, 256], F16, name="bankA", bufs=6)
                nc.tensor.matmul(pyt[:, 0:128], ych[:, 0:128], id_sb[:, :],
                                 is_transpose=True, start=True, stop=True,
                                 skip_group_check=True)
                nc.tensor.matmul(pyt[:, 128:256], ych[:, 128:256],
                                 id_sb[:, :], is_transpose=True,
                                 start=False, stop=True,
                                 skip_group_check=True)
                nc.scalar.copy(
                    yt_all[:, :].rearrange("p (a t) -> p a t", a=2)
                    [:, :, ci * 128:(ci + 1) * 128],
                    pyt[:, :].rearrange("p (a c) -> p a c", a=2))

            def favor_C2(ci):
                # ---- phase 4 for this token tile ----
                osb = cpy.tile([128, 1024], F16, name="osb")
                last = ci == NT - 1
                for ni in range(2):
                    nsl = slice(ni * 512, (ni + 1) * 512)
                    pp = bankA()
                    for ci2 in range(2):
                        nc.tensor.matmul(
                            pp[:, :],
                            yt_all[:, ci2 * T + ci * 128:
                                   ci2 * T + (ci + 1) * 128],
                            wp_sb[ci2][:, nsl],
                            start=(ci2 == 0), stop=(ci2 == 1))
                    if (ni == 0) != last:
                        nc.scalar.copy(osb[:, nsl], pp[:, :])
                    else:
                        nc.vector.tensor_copy(osb[:, nsl], pp[:, :])
                # one contiguous [128, 1024] drain per token tile
                keng = nc.scalar if last else nc.sync
                keng.dma_start(
                    out=outp[ci * 128:(ci + 1) * 128, :], in_=osb[:, :])

            # ---- program order ----
            # Token half 0 (chunks 0-3) is fully independent of half 1, so
            # its FAVOR chunks are interleaved with half 1's qkv/exp phases:
            # the favor pipeline starts as soon as half 0 is projected while
            # x slabs 4-7 are still arriving.
            phase1_interleaved(0)
            e_k_pair(0, 0)
            e_q_pair(0, 0)
            e_k_pair(1, 0)
            e_q_pair(1, 0)
            for ci in range(4):
                chunk_pre_pk(ci, 0)
                chunk_pre_pk(ci, 1)
            v_group(0)
            chunk_pre_vh(0, 0)
            chunk_pre_vh(0, 1)
            qk_group(2, 1)
            favor_A(0)
            v_group(1)
            chunk_pre_vh(1, 0)
            chunk_pre_vh(1, 1)
            qk_group(3, 1)
            favor_A(1)
            favor_B(0)
            v_group(2)
            chunk_pre_vh(2, 0)
            chunk_pre_vh(2, 1)
            chunk_pre_pk(4, 0)
            chunk_pre_pk(4, 1)
            e_k_pair(0, 1)
            e_k_pair(1, 1)
            favor_A(2)
            favor_B(1)
            favor_C1(0)
            v_group(3)
            chunk_pre_vh(3, 0)
            chunk_pre_vh(3, 1)
            qk_group(0, 1)
            qk_group(1, 1)
            favor_A(3)
            favor_B(2)
            favor_C1(1)
            favor_C2(0)
            e_q_pair(0, 1)
            e_q_pair(1, 1)
            for ci in range(5, 8):
                chunk_pre_pk(ci, 0)
                chunk_pre_pk(ci, 1)
            favor_B(3)
            favor_C1(2)
            favor_C2(1)
            for ti in range(4, 8):
                v_group(ti)
            for ci in range(4, 8):
                chunk_pre_vh(ci, 0)
                chunk_pre_vh(ci, 1)
            favor_C1(3)
            favor_C2(2)
            favor_A(4)
            favor_C2(3)
            favor_A(5)
            favor_B(4)
            favor_A(6)
            favor_B(5)
            favor_C1(4)
            favor_A(7)
            favor_B(6)
            favor_C1(5)
            favor_C2(4)
            favor_B(7)
            favor_C1(6)
            favor_C2(5)
            favor_C1(7)
            favor_C2(6)
            favor_C2(7)

    _split_waits(nc)
    return nc


_NC_CACHE = None


def _get_nc():
    global _NC_CACHE
    if _NC_CACHE is None:
        _NC_CACHE = build_bass()
    return _NC_CACHE


def _img8(w):
    # [1024, n] -> [128, 8*n] with 128-row blocks laid side by side
    n = w.shape[1]
    return np.ascontiguousarray(
        w.reshape(8, 128, n).transpose(1, 0, 2).reshape(128, 8 * n))


def kernel(x, W_attn, b_attn, W_proj, b_proj, omega):
    from concourse.bass_utils import run_bass_kernel_spmd

    x = np.asarray(x, dtype=np.float32)
    W_attn = np.asarray(W_attn, dtype=np.float32)
    b_attn = np.asarray(b_attn, dtype=np.float32)
    W_proj = np.asarray(W_proj, dtype=np.float32)
    b_proj = np.asarray(b_proj, dtype=np.float32)
    omega = np.asarray(omega, dtype=np.float32)

    B = x.shape[0]
    scale = 1.0 / math.sqrt(D)
    omega2 = np.concatenate([omega, omega], axis=0)
    omnsq = np.zeros((128, 129), np.float32)
    omnsq[0:64, 0:128] = omega
    omnsq[64:128, 128] = -0.5
    omnsq2 = np.zeros((128, 129), np.float32)
    omnsq2[64:128, 0:128] = omega
    omnsq2[0:64, 128] = -0.5
    maskT = np.triu(np.ones((128, 128), np.float32))
    ident = np.eye(128, dtype=np.float32)
    c16_base = np.zeros((128, C16_W), np.float16)
    c16_base[:, C16_OM:C16_OM + 128] = omega2
    c16_base[:, C16_ON:C16_ON + 129] = omnsq
    c16_base[:, C16_ON2:C16_ON2 + 129] = omnsq2
    c16_base[:, C16_MK:C16_MK + 128] = maskT
    c16_base[:, C16_ID:C16_ID + 128] = ident
    c16_base = np.ascontiguousarray(c16_base)

    # x images: 8 slabs of (ki pair, token half), token half 0 first
    ximgs = []
    for b in range(B):
        xT = np.ascontiguousarray(x[b].T).astype(np.float16)
        Xr = xT.reshape(8, 128, 1024)
        slabs = []
        for j in range(8):
            kp, th = (j % 4) * 2, j // 4
            s = Xr[kp:kp + 2, :, th * 512:(th + 1) * 512]   # [2,128,512]
            slabs.append(s.transpose(1, 0, 2).reshape(128, 1024))
        ximgs.append(np.ascontiguousarray(np.concatenate(slabs, axis=1)))

    # b_attn is zeros by the problem spec, so qkv biases are dropped on
    # device; b_proj is added on the host below.
    in_maps = []
    for core in range(8):
        b, g = core // 4, core % 4
        ch0 = g * HPC * D
        wq = (W_attn[:, ch0:ch0 + HPC * D] * scale).astype(np.float16)
        wk = (W_attn[:, C + ch0:C + ch0 + HPC * D] * scale).astype(np.float16)
        wv_ = W_attn[:, 2 * C + ch0:2 * C + ch0 + HPC * D].astype(np.float16)
        wp_ = W_proj[ch0:ch0 + HPC * D, :].astype(np.float16)
        in_maps.append({
            "ximg": ximgs[b],
            "wqkk": _img8(wk), "wqkq": _img8(wq), "wvimg": _img8(wv_),
            "wpimg": np.ascontiguousarray(
                wp_.reshape(2, 128, 1024).transpose(1, 0, 2)
                .reshape(128, 2048)),
            "consts16": c16_base,
        })

    nc = _get_nc()
    res = run_bass_kernel_spmd(nc, in_maps, list(range(8)))

    out = np.zeros((B, T, C), dtype=np.float32)
    for core in range(8):
        out[core // 4] += res.results[core]["outp"]
    out += b_proj[None, None, :]
    return out

